# revision 8
# baseline (speedup 1.0000x reference)
"""Trainium2 Bass kernel for the DIN sparse-attention module, v2.

Restructure vs v1 (same math, new schedule):
  - Sigmoids -> Tanh via sigma(x) = (1+tanh(x/2))/2 with host-side weight
    folding, so every activation {Tanh, Tanh, Exp} is served by ONE act
    table (exp_and_others): kills the 64 per-group table reloads.
  - Layer-1 tmp4 built with 2 batch-wide tensor_tensor ops (F.T*q + U)
    instead of 4 tensor_scalars per group.
  - Layer-2 activations packed 2 groups per instruction (PSUM strips at
    partitions 0:40 and 64:104).
  - Layer-3 split into MM_a (w3 on h2) + MM_b (indicator/penalty/mask rows)
    accumulating in PSUM; mask is folded in via a +BIG row against
    (mask-1), so the softmax tail needs no mask multiply.
  - exp packed 4 groups per instruction: ps3 holds 4 strips of 32 rows
    (per-group penalties select exact in-strip rows); two exp'd quads are
    strip-summed back to [32, F*4] with one small matmul pair (Msum), which
    replaces v1's 64 SBUF->SBUF collect DMAs entirely.
  - All inputs loaded with 3 batched DMAs; consts with 2.

Sharding: pure data parallel, 8 batches per core across 8 cores.
"""

import numpy as np
import ml_dtypes

import concourse.bass as bass
import concourse.bacc as bacc
import concourse.tile as tile
from concourse import mybir
from concourse.bass_utils import run_bass_kernel_spmd
from concourse.masks import make_identity

B, Q, F, D = 64, 32, 128, 128
N_CORES = 8
BPC = B // N_CORES          # batches per core
GQ = 4                      # queries per group
N_GROUPS = Q // GQ          # 8 groups per batch
PACK = 4                    # batches packed per tail pass
BIG = 1.0e5

f32 = mybir.dt.float32
bf16 = mybir.dt.bfloat16
i32 = mybir.dt.int32
AF = mybir.ActivationFunctionType
ALU = mybir.AluOpType

# bf16 const grid column layout
C_WP = 0          # [128, 80]
C_WC = 80         # [128, 80]
C_MU = 160        # [128, 128]
C_W2 = 288        # [80, 64] (w2/2 zero-padded to 64 out-cols)
C_W3 = 352        # [40, 32]
C_MS = 384        # [128, 32]
C_LB0 = 416       # [5, 128]
C_LB1 = 544       # [5, 128]
C_W3B = 672       # [rows 64:104, 32] second copy of w3/2 for the upper pair
C_END = 704


def _bcast(ap: bass.AP, reps: int, inner: int = 1) -> bass.AP:
    """Insert a step-0 broadcast dim before the last `inner` free dims."""
    dims = [list(d) for d in ap.ap]
    pos = len(dims) - inner
    new = dims[:pos] + [[0, reps]] + dims[pos:]
    return bass.AP(tensor=ap.tensor, offset=ap.offset, ap=new)


def _split_free(ap: bass.AP, outer: int, inner: int) -> bass.AP:
    """View a [..., outer*inner] AP as [..., outer, inner] (contig inner)."""
    dims = [list(d) for d in ap.ap]
    step, cnt = dims[-1]
    assert cnt == outer * inner and step == 1
    new = dims[:-1] + [[inner, outer], [1, inner]]
    return bass.AP(tensor=ap.tensor, offset=ap.offset, ap=new)


def build_program(reps: int = 1):
    nc = bacc.Bacc("TRN2", target_bir_lowering=False, debug=False)

    query_t = nc.dram_tensor("query", [BPC, Q, D], f32, kind="ExternalInput")
    facts_t = nc.dram_tensor("facts", [BPC, F, D], f32, kind="ExternalInput")
    mask_t = nc.dram_tensor("mask", [BPC, F], i32, kind="ExternalInput")
    cbf_t = nc.dram_tensor("cbf", [128, C_END], bf16, kind="ExternalInput")
    indm_t = nc.dram_tensor("indm", [2, 5, 512], bf16, kind="ExternalInput")
    cf32_t = nc.dram_tensor("cf32", [128, 2], f32, kind="ExternalInput")
    out_t = nc.dram_tensor("out", [BPC, Q], f32, kind="ExternalOutput")

    with tile.TileContext(nc) as tc:
        with (
            tc.tile_pool(name="consts", bufs=1) as consts,
            tc.tile_pool(name="batch", bufs=3) as batch_pool,
            tc.tile_pool(name="tmp4p", bufs=2) as tmp4_pool,
            tc.tile_pool(name="h1p", bufs=4) as h1_pool,
            tc.tile_pool(name="h2p", bufs=3) as h2_pool,
            tc.tile_pool(name="eqp", bufs=3) as eq_pool,
            tc.tile_pool(name="packp", bufs=2) as pack_pool,
            tc.tile_pool(name="ps1", bufs=2, space="PSUM") as ps1_pool,
            tc.tile_pool(name="ps2", bufs=2, space="PSUM") as ps2_pool,
            tc.tile_pool(name="ps3", bufs=2, space="PSUM") as ps3_pool,
            tc.tile_pool(name="psT", bufs=1, space="PSUM") as psT_pool,
            tc.tile_pool(name="psG", bufs=1, space="PSUM") as psG_pool,
        ):
            # ---------------- constants ----------------
            identity = consts.tile([128, 128], f32)
            make_identity(nc, identity)

            cbf = consts.tile([128, C_END], bf16)
            nc.sync.dma_start(out=cbf, in_=cbf_t.ap())
            cf = consts.tile([128, 2], f32)
            nc.sync.dma_start(out=cf, in_=cf32_t.ap())

            wP = cbf[:, C_WP : C_WP + 80]
            wC = cbf[:, C_WC : C_WC + 80]
            mu_bf = cbf[:, C_MU : C_MU + 128]
            w2h = cbf[0:80, C_W2 : C_W2 + 64]
            w3rep = cbf[0:40, C_W3 : C_W3 + 32]
            w3rep_hi = cbf[64:104, C_W3B : C_W3B + 32]
            msum = cbf[:, C_MS : C_MS + 32]
            lhsTB = [cbf[0:5, C_LB0 : C_LB0 + 128], cbf[0:5, C_LB1 : C_LB1 + 128]]
            b1h = cf[0:80, 0:1]
            b2h = cf[:, 1:2]

            # rhs tiles for MM_b: rows 1:5 = indicator pattern (host),
            # row 0 = per-batch (mask-1); two tiles alternate per batch.
            indmask = []
            for t in range(2):
                im = consts.tile([5, 512], bf16, tag=f"im{t}")
                nc.sync.dma_start(out=im, in_=indm_t.ap()[t])
                indmask.append(im)

            # ---------------- batched input loads ----------------
            facts_sb = consts.tile([128, BPC, D], f32)   # [f, b, d]
            nc.sync.dma_start(
                out=facts_sb,
                in_=bass.AP(
                    tensor=facts_t, offset=0,
                    ap=[[D, F], [F * D, BPC], [1, D]],
                ),
            )
            query_sb = consts.tile([Q, BPC, D], f32)     # [q, b, d]
            nc.sync.dma_start(
                out=query_sb,
                in_=bass.AP(
                    tensor=query_t, offset=0,
                    ap=[[D, Q], [Q * D, BPC], [1, D]],
                ),
            )
            mask_i = consts.tile([1, BPC, F], i32)
            nc.sync.dma_start(
                out=mask_i,
                in_=bass.AP(
                    tensor=mask_t, offset=0,
                    ap=[[BPC * F, 1], [F, BPC], [1, F]],
                ),
            )
            mask_f = consts.tile([1, BPC, F], f32)
            nc.vector.tensor_copy(mask_f, mask_i)

            # ---------------- main loop ----------------
            for _rep in range(reps):
              for pack in range(BPC // PACK):
                G_ps = psG_pool.tile([128, F], f32)
                e_pack = pack_pool.tile([128, GQ, F], bf16)

                for b_l in range(PACK):
                    b = pack * PACK + b_l

                    # transposes + per-batch precomputes
                    T_ps = psT_pool.tile([128, F], f32, tag="tps")
                    nc.tensor.transpose(T_ps, facts_sb[:, b, :], identity)
                    F_Tb = batch_pool.tile([D, F], bf16)
                    nc.vector.tensor_copy(F_Tb, T_ps)

                    T2_ps = psT_pool.tile([128, Q], f32, tag="tps")
                    nc.tensor.transpose(
                        T2_ps, query_sb[:, b, :], identity[0:Q, 0:Q]
                    )
                    Q_Tb = batch_pool.tile([D, Q], bf16)
                    nc.vector.tensor_copy(Q_Tb, T2_ps)
                    Q_Tf = batch_pool.tile([D, Q], f32)
                    nc.vector.tensor_copy(Q_Tf, T2_ps)

                    # U = M_u @ Q_T folds the W_A term into tmp4
                    U_ps = psT_pool.tile([128, Q], f32, tag="tps")
                    nc.tensor.matmul(U_ps, mu_bf, Q_Tb, start=True, stop=True)
                    U_f = batch_pool.tile([D, Q], f32)
                    nc.vector.tensor_copy(U_f, U_ps)

                    # G[q, f] = <query_q, facts_f>
                    nc.tensor.matmul(
                        G_ps[32 * b_l : 32 * b_l + 32, :],
                        Q_Tb,
                        F_Tb,
                        start=True,
                        stop=True,
                        tile_position=(0, 32 * b_l),
                    )

                    # mask row for MM_b: (mask-1) tiled over the 4 q-blocks
                    im = indmask[b % 2]
                    row4 = im[0:1, :]
                    out3 = bass.AP(
                        tensor=row4.tensor, offset=row4.offset,
                        ap=[list(row4.ap[0]), [F, GQ], [1, F]],
                    )
                    nc.vector.tensor_scalar(
                        out3, _bcast(mask_f[:, b, :], GQ), 1.0, None,
                        op0=ALU.subtract,
                    )

                    # tmp4[d, q, f] = F_T[d,f]*Q_T[d,q] + U[d,q]
                    tmp4 = tmp4_pool.tile([D, Q, F], bf16)
                    for q in range(Q):
                        nc.vector.tensor_scalar(
                            tmp4[:, q],
                            F_Tb,
                            Q_Tf[:, q : q + 1],
                            U_f[:, q : q + 1],
                            op0=ALU.mult,
                            op1=ALU.add,
                        )

                    for quad in range(2):            # 4 groups per quad
                        ps3q = ps3_pool.tile([128, GQ * F], f32, tag="ps3")
                        for pair in range(2):        # 2 groups per pair
                            ps2t = ps2_pool.tile([128, GQ * F], f32)
                            for gg in range(2):
                                g = quad * 4 + pair * 2 + gg
                                ps1 = ps1_pool.tile([80, GQ * F], f32)
                                nc.tensor.matmul(
                                    ps1,
                                    wP,
                                    tmp4[:, GQ * g : GQ * (g + 1), :].rearrange(
                                        "d g f -> d (g f)"
                                    ),
                                    start=True,
                                    stop=False,
                                )
                                nc.tensor.matmul(
                                    ps1, wC, _bcast(F_Tb, GQ),
                                    start=False, stop=True,
                                )
                                h1 = h1_pool.tile([80, GQ * F], bf16)
                                nc.scalar.activation(
                                    h1, ps1, AF.Tanh, bias=b1h, scale=0.5
                                )
                                nc.tensor.matmul(
                                    ps2t[64 * gg : 64 * gg + 64, :],
                                    w2h,
                                    h1,
                                    start=True,
                                    stop=True,
                                    tile_position=(0, 64 * gg),
                                )
                            # one tanh for both groups of the pair
                            h2pair = h2_pool.tile([104, GQ * F], bf16)
                            nc.scalar.activation(
                                h2pair, ps2t[0:104, :], AF.Tanh,
                                bias=b2h[0:104, :], scale=0.5,
                            )
                            for gg in range(2):
                                s = pair * 2 + gg
                                nc.tensor.matmul(
                                    ps3q[32 * s : 32 * s + 32, :],
                                    lhsTB[quad][:, 32 * s : 32 * s + 32],
                                    im,
                                    start=True,
                                    stop=False,
                                    tile_position=(0, 32 * s),
                                )
                                nc.tensor.matmul(
                                    ps3q[32 * s : 32 * s + 32, :],
                                    w3rep if gg == 0 else w3rep_hi,
                                    h2pair[64 * gg : 64 * gg + 40, :],
                                    start=False,
                                    stop=True,
                                    tile_position=(64 * gg, 32 * s),
                                )
                        # one exp for the whole quad
                        eq = eq_pool.tile([128, GQ * F], bf16, tag=f"eq{quad}")
                        nc.scalar.activation(eq, ps3q, AF.Exp)
                        if quad == 0:
                            eqA = eq
                        else:
                            eqB = eq

                    # strip-sum both quads back to [32, 512] and place into
                    # the pack tile at partition strip 32*b_l
                    ps_e = ps3_pool.tile([128, GQ * F], f32, tag="ps3")
                    nc.tensor.matmul(
                        ps_e[0:32, :], msum, eqA, start=True, stop=False
                    )
                    nc.tensor.matmul(
                        ps_e[0:32, :], msum, eqB, start=False, stop=True
                    )
                    nc.vector.tensor_copy(
                        e_pack[32 * b_l : 32 * b_l + 32, :, :].rearrange(
                            "p g f -> p (g f)"
                        ),
                        ps_e[0:32, :],
                    )

                # ---------------- pack tail ----------------
                esum = pack_pool.tile([128, 1], f32)
                nc.vector.tensor_reduce(
                    esum, e_pack, axis=mybir.AxisListType.XY, op=ALU.add
                )
                G_bf = pack_pool.tile([128, F], bf16)
                nc.vector.tensor_copy(G_bf, G_ps)
                eg = pack_pool.tile([128, GQ, F], bf16)
                wsum = pack_pool.tile([128, 1], f32)
                nc.vector.tensor_mul(eg, e_pack, _bcast(G_bf, GQ))
                nc.vector.tensor_reduce(
                    wsum, eg, axis=mybir.AxisListType.XY, op=ALU.add
                )
                rsum = pack_pool.tile([128, 1], f32)
                nc.vector.reciprocal(rsum, esum)
                outcol = pack_pool.tile([128, 1], f32)
                nc.vector.tensor_mul(outcol, wsum, rsum)
                nc.sync.dma_start(
                    out=bass.AP(
                        tensor=out_t, offset=128 * pack, ap=[[1, 128], [1, 1]]
                    ),
                    in_=outcol,
                )

    nc.compile()
    return nc


_CACHED = {}


def _get_program(reps: int = 1):
    if reps not in _CACHED:
        _CACHED[reps] = build_program(reps)
    return _CACHED[reps]


def _host_consts(inputs):
    w1 = np.asarray(inputs["w1"], np.float32)
    b1 = np.asarray(inputs["b1"], np.float32)
    w2 = np.asarray(inputs["w2"], np.float32)
    b2 = np.asarray(inputs["b2"], np.float32)
    w3 = np.asarray(inputs["w3"], np.float32)

    W_A = w1[0:128] + w1[256:384]
    W_C = w1[128:256] - w1[256:384]
    W_P = w1[384:512]

    gram = (W_P.T @ W_P).astype(np.float64)
    M_u = (W_P @ np.linalg.solve(gram, W_A.T.astype(np.float64))).astype(
        np.float32
    )

    # sigmoid -> tanh folding: h1 = (1+t1)/2 with t1 = tanh(z1/2), so
    # z2 = w2.T h1 + b2 = (w2/2).T t1 + (b2 + w2.T 1 / 2); same for layer 3.
    w2half = 0.5 * w2
    b2eff = b2 + 0.5 * w2.sum(axis=0)
    w3half = 0.5 * w3[:, 0]

    cbf = np.zeros((128, C_END), np.float32)
    cbf[:, C_WP : C_WP + 80] = W_P
    cbf[:, C_WC : C_WC + 80] = W_C
    cbf[:, C_MU : C_MU + 128] = M_u.T
    cbf[0:80, C_W2 : C_W2 + 64] = np.concatenate(
        [w2half, np.zeros((80, 24), np.float32)], axis=1
    )
    cbf[0:40, C_W3 : C_W3 + 32] = w3half[:, None]
    cbf[64:104, C_W3B : C_W3B + 32] = w3half[:, None]
    # Msum[p, i] = 1 iff p % 32 == i  (strip-sum)
    p = np.arange(128)
    cbf[:, C_MS : C_MS + 32] = (p[:, None] % 32 == np.arange(32)[None, :])
    # per-quad penalty lhsT: rows j 0:4 -> 0 at in-strip row i == 4g+j of
    # strip s (group g = 4*quad + s), else -BIG; row 4 = +BIG (mask fold)
    for qd, col in ((0, C_LB0), (1, C_LB1)):
        lb = np.full((5, 128), -BIG, np.float32)
        for s in range(4):
            g = 4 * qd + s
            for j in range(GQ):
                i = 4 * g + j
                lb[1 + j, 32 * s + i] = 0.0
        lb[0, :] = BIG  # mask row (rhs row 0 = mask-1)
        cbf[0:5, col : col + 128] = lb
    indm = np.zeros((2, 5, 512), np.float32)
    for j in range(GQ):
        indm[:, 1 + j, j * F : (j + 1) * F] = 1.0

    cf32 = np.zeros((128, 2), np.float32)
    cf32[0:80, 0] = 0.5 * b1
    cf32[0:40, 1] = 0.5 * b2eff
    cf32[64:104, 1] = 0.5 * b2eff

    return (
        np.ascontiguousarray(cbf.astype(ml_dtypes.bfloat16)),
        np.ascontiguousarray(cf32),
        np.ascontiguousarray(indm.astype(ml_dtypes.bfloat16)),
    )


def _make_in_maps(inputs):
    query = np.ascontiguousarray(np.asarray(inputs["query"], np.float32))
    facts = np.ascontiguousarray(np.asarray(inputs["facts"], np.float32))
    mask = np.ascontiguousarray(np.asarray(inputs["mask"], np.int32))
    cbf, cf32, indm = _host_consts(inputs)

    in_maps = []
    for c in range(N_CORES):
        sl = slice(c * BPC, (c + 1) * BPC)
        in_maps.append(
            {
                "query": np.ascontiguousarray(query[sl]),
                "facts": np.ascontiguousarray(facts[sl]),
                "mask": np.ascontiguousarray(mask[sl]),
                "cbf": cbf,
                "cf32": cf32,
                "indm": indm,
            }
        )
    return in_maps


def run_traced(inputs, trace=False, reps=1):
    nc = _get_program(reps)
    res = run_bass_kernel_spmd(
        nc, _make_in_maps(inputs), core_ids=list(range(N_CORES)), trace=trace
    )
    out = np.concatenate(
        [res.results[c]["out"] for c in range(N_CORES)], axis=0
    )
    return out.astype(np.float32), res.exec_time_ns


def kernel(**inputs) -> np.ndarray:
    out, _ = run_traced(inputs, trace=False)
    return out


# revision 11
# speedup vs baseline: 1.8483x; 1.8483x over previous
"""Trainium2 Bass kernel for the DIN sparse-attention module, v2.

Restructure vs v1 (same math, new schedule):
  - Sigmoids -> Tanh via sigma(x) = (1+tanh(x/2))/2 with host-side weight
    folding, so every activation {Tanh, Tanh, Exp} is served by ONE act
    table (exp_and_others): kills the 64 per-group table reloads.
  - Layer-1 tmp4 built with 2 batch-wide tensor_tensor ops (F.T*q + U)
    instead of 4 tensor_scalars per group.
  - Layer-2 activations packed 2 groups per instruction (PSUM strips at
    partitions 0:40 and 64:104).
  - Layer-3 split into MM_a (w3 on h2) + MM_b (indicator/penalty/mask rows)
    accumulating in PSUM; mask is folded in via a +BIG row against
    (mask-1), so the softmax tail needs no mask multiply.
  - exp packed 4 groups per instruction: ps3 holds 4 strips of 32 rows
    (per-group penalties select exact in-strip rows); two exp'd quads are
    strip-summed back to [32, F*4] with one small matmul pair (Msum), which
    replaces v1's 64 SBUF->SBUF collect DMAs entirely.
  - All inputs loaded with 3 batched DMAs; consts with 2.

Sharding: pure data parallel, 8 batches per core across 8 cores.
"""

import numpy as np
import ml_dtypes

import concourse.bass as bass
import concourse.bacc as bacc
import concourse.tile as tile
from concourse import mybir
from concourse.bass_utils import run_bass_kernel_spmd
from concourse.masks import make_identity

B, Q, F, D = 64, 32, 128, 128
N_CORES = 8
BPC = B // N_CORES          # batches per core
GQ = 4                      # queries per group
N_GROUPS = Q // GQ          # 8 groups per batch
PACK = 4                    # batches packed per tail pass
BIG = 1.0e5

f32 = mybir.dt.float32
bf16 = mybir.dt.bfloat16
i32 = mybir.dt.int32
AF = mybir.ActivationFunctionType
ALU = mybir.AluOpType

# bf16 const grid column layout
C_WP = 0          # [128, 80]
C_WC = 80         # [128, 80]
C_MU = 160        # [128, 128]
C_W2 = 288        # [80, 64] (w2/2 zero-padded to 64 out-cols)
C_W3 = 352        # [40, 32]
C_MS = 384        # [128, 32]
C_W3B = 416       # [rows 64:104, 32] second copy of w3/2 for the upper pair
C_P0 = 448        # [128, 512] 0/1 validity pattern, quad 0
C_P1 = 960        # [128, 512] 0/1 validity pattern, quad 1
C_END = 1472


def _bcast(ap: bass.AP, reps: int, inner: int = 1) -> bass.AP:
    """Insert a step-0 broadcast dim before the last `inner` free dims."""
    dims = [list(d) for d in ap.ap]
    pos = len(dims) - inner
    new = dims[:pos] + [[0, reps]] + dims[pos:]
    return bass.AP(tensor=ap.tensor, offset=ap.offset, ap=new)


def _split_free(ap: bass.AP, outer: int, inner: int) -> bass.AP:
    """View a [..., outer*inner] AP as [..., outer, inner] (contig inner)."""
    dims = [list(d) for d in ap.ap]
    step, cnt = dims[-1]
    assert cnt == outer * inner and step == 1
    new = dims[:-1] + [[inner, outer], [1, inner]]
    return bass.AP(tensor=ap.tensor, offset=ap.offset, ap=new)


def build_program(reps: int = 1):
    nc = bacc.Bacc("TRN2", target_bir_lowering=False, debug=False)

    query_t = nc.dram_tensor("query", [BPC, Q, D], f32, kind="ExternalInput")
    facts_t = nc.dram_tensor("facts", [BPC, F, D], f32, kind="ExternalInput")
    mask_t = nc.dram_tensor("mask", [BPC, F], i32, kind="ExternalInput")
    cbf_t = nc.dram_tensor("cbf", [128, C_END], bf16, kind="ExternalInput")
    cf32_t = nc.dram_tensor("cf32", [128, 2], f32, kind="ExternalInput")
    out_t = nc.dram_tensor("out", [BPC, Q], f32, kind="ExternalOutput")

    with tile.TileContext(nc) as tc:
        with (
            tc.tile_pool(name="consts", bufs=1) as consts,
            tc.tile_pool(name="batch", bufs=3) as batch_pool,
            tc.tile_pool(name="tmp4p", bufs=2) as tmp4_pool,
            tc.tile_pool(name="h1p", bufs=4) as h1_pool,
            tc.tile_pool(name="h2p", bufs=3) as h2_pool,
            tc.tile_pool(name="eqp", bufs=3) as eq_pool,
            tc.tile_pool(name="packp", bufs=2) as pack_pool,
            tc.tile_pool(name="ps1", bufs=2, space="PSUM") as ps1_pool,
            tc.tile_pool(name="ps2", bufs=2, space="PSUM") as ps2_pool,
            tc.tile_pool(name="ps3", bufs=2, space="PSUM") as ps3_pool,
            tc.tile_pool(name="psT", bufs=1, space="PSUM") as psT_pool,
            tc.tile_pool(name="psG", bufs=1, space="PSUM") as psG_pool,
        ):
            # ---------------- constants ----------------
            identity = consts.tile([128, 128], f32)
            make_identity(nc, identity)

            cbf = consts.tile([128, C_END], bf16)
            nc.sync.dma_start(out=cbf, in_=cbf_t.ap())
            cf = consts.tile([128, 2], f32)
            nc.sync.dma_start(out=cf, in_=cf32_t.ap())

            wP = cbf[:, C_WP : C_WP + 80]
            wC = cbf[:, C_WC : C_WC + 80]
            mu_bf = cbf[:, C_MU : C_MU + 128]
            w2h = cbf[0:80, C_W2 : C_W2 + 64]
            w3rep = cbf[0:40, C_W3 : C_W3 + 32]
            w3rep_hi = cbf[64:104, C_W3B : C_W3B + 32]
            msum = cbf[:, C_MS : C_MS + 32]
            pen01 = [cbf[:, C_P0 : C_P0 + 512], cbf[:, C_P1 : C_P1 + 512]]
            b1h = cf[0:80, 0:1]
            b2h = cf[:, 1:2]


            # ---------------- batched input loads ----------------
            facts_sb = consts.tile([128, BPC, D], f32)   # [f, b, d]
            nc.sync.dma_start(
                out=facts_sb,
                in_=bass.AP(
                    tensor=facts_t, offset=0,
                    ap=[[D, F], [F * D, BPC], [1, D]],
                ),
            )
            query_sb = consts.tile([Q, BPC, D], f32)     # [q, b, d]
            nc.sync.dma_start(
                out=query_sb,
                in_=bass.AP(
                    tensor=query_t, offset=0,
                    ap=[[D, Q], [Q * D, BPC], [1, D]],
                ),
            )

            # ---------------- main loop ----------------
            for _rep in range(reps):
              for pack in range(BPC // PACK):
                G_ps = psG_pool.tile([128, F], f32)
                e_pack = pack_pool.tile([128, GQ, F], bf16)
                mask_i = pack_pool.tile([128, F], i32)

                for b_l in range(PACK):
                    b = pack * PACK + b_l

                    # transposes + per-batch precomputes
                    T_ps = psT_pool.tile([128, F], f32, tag="tps")
                    nc.tensor.transpose(T_ps, facts_sb[:, b, :], identity)
                    F_Tb = batch_pool.tile([D, F], bf16)
                    nc.vector.tensor_copy(F_Tb, T_ps)

                    T2_ps = psT_pool.tile([128, Q], f32, tag="tps")
                    nc.tensor.transpose(
                        T2_ps, query_sb[:, b, :], identity[0:Q, 0:Q]
                    )
                    Q_Tb = batch_pool.tile([D, Q], bf16)
                    nc.vector.tensor_copy(Q_Tb, T2_ps)
                    Q_Tf = batch_pool.tile([D, Q], f32)
                    nc.vector.tensor_copy(Q_Tf, T2_ps)

                    # U = M_u @ Q_T folds the W_A term into tmp4
                    U_ps = psT_pool.tile([128, Q], f32, tag="tps")
                    nc.tensor.matmul(U_ps, mu_bf, Q_Tb, start=True, stop=True)
                    U_f = batch_pool.tile([D, Q], f32)
                    nc.vector.tensor_copy(U_f, U_ps)

                    # G[q, f] = <query_q, facts_f>
                    nc.tensor.matmul(
                        G_ps[32 * b_l : 32 * b_l + 32, :],
                        Q_Tb,
                        F_Tb,
                        start=True,
                        stop=True,
                        tile_position=(0, 32 * b_l),
                    )

                    # mask rows for the tail multiply (broadcast over queries)
                    nc.sync.dma_start(
                        out=mask_i[32 * b_l : 32 * b_l + 32, :],
                        in_=bass.AP(
                            tensor=mask_t, offset=b * F, ap=[[0, Q], [1, F]]
                        ),
                    )

                    # tmp4[d, q, f] = F_T[d,f]*Q_T[d,q] + U[d,q]
                    tmp4 = tmp4_pool.tile([D, Q, F], bf16)
                    for q in range(Q):
                        nc.vector.tensor_scalar(
                            tmp4[:, q],
                            F_Tb,
                            Q_Tf[:, q : q + 1],
                            U_f[:, q : q + 1],
                            op0=ALU.mult,
                            op1=ALU.add,
                        )

                    for quad in range(2):            # 4 groups per quad
                        ps3q = ps3_pool.tile([128, GQ * F], f32, tag="ps3")
                        for pair in range(2):        # 2 groups per pair
                            ps2t = ps2_pool.tile([128, GQ * F], f32)
                            for gg in range(2):
                                g = quad * 4 + pair * 2 + gg
                                ps1 = ps1_pool.tile([80, GQ * F], f32)
                                nc.tensor.matmul(
                                    ps1,
                                    wP,
                                    tmp4[:, GQ * g : GQ * (g + 1), :].rearrange(
                                        "d g f -> d (g f)"
                                    ),
                                    start=True,
                                    stop=False,
                                )
                                nc.tensor.matmul(
                                    ps1, wC, _bcast(F_Tb, GQ),
                                    start=False, stop=True,
                                )
                                h1 = h1_pool.tile([80, GQ * F], bf16)
                                nc.scalar.activation(
                                    h1, ps1, AF.Tanh, bias=b1h, scale=0.5
                                )
                                nc.tensor.matmul(
                                    ps2t[64 * gg : 64 * gg + 64, :],
                                    w2h,
                                    h1,
                                    start=True,
                                    stop=True,
                                    tile_position=(0, 64 * gg),
                                )
                            # one tanh for both groups of the pair
                            h2pair = h2_pool.tile([104, GQ * F], bf16)
                            nc.scalar.activation(
                                h2pair, ps2t[0:104, :], AF.Tanh,
                                bias=b2h[0:104, :], scale=0.5,
                            )
                            for gg in range(2):
                                s = pair * 2 + gg
                                nc.tensor.matmul(
                                    ps3q[32 * s : 32 * s + 32, :],
                                    w3rep if gg == 0 else w3rep_hi,
                                    h2pair[64 * gg : 64 * gg + 40, :],
                                    start=True,
                                    stop=True,
                                    tile_position=(64 * gg, 32 * s),
                                )
                        # one exp for the whole quad
                        eq = eq_pool.tile([128, GQ * F], bf16, tag=f"eq{quad}")
                        nc.scalar.activation(eq, ps3q, AF.Exp)
                        nc.vector.tensor_mul(eq, eq, pen01[quad])
                        if quad == 0:
                            eqA = eq
                        else:
                            eqB = eq

                    # strip-sum both quads back to [32, 512] and place into
                    # the pack tile at partition strip 32*b_l
                    ps_e = ps3_pool.tile([128, GQ * F], f32, tag="ps3")
                    nc.tensor.matmul(
                        ps_e[0:32, :], msum, eqA, start=True, stop=False
                    )
                    nc.tensor.matmul(
                        ps_e[0:32, :], msum, eqB, start=False, stop=True
                    )
                    nc.vector.tensor_copy(
                        e_pack[32 * b_l : 32 * b_l + 32, :, :].rearrange(
                            "p g f -> p (g f)"
                        ),
                        ps_e[0:32, :],
                    )

                # ---------------- pack tail ----------------
                mask_bf = pack_pool.tile([128, F], bf16)
                nc.vector.tensor_copy(mask_bf, mask_i)
                nc.vector.tensor_mul(e_pack, e_pack, _bcast(mask_bf, GQ))
                esum = pack_pool.tile([128, 1], f32)
                nc.vector.tensor_reduce(
                    esum, e_pack, axis=mybir.AxisListType.XY, op=ALU.add
                )
                G_bf = pack_pool.tile([128, F], bf16)
                nc.vector.tensor_copy(G_bf, G_ps)
                eg = pack_pool.tile([128, GQ, F], bf16)
                wsum = pack_pool.tile([128, 1], f32)
                nc.vector.tensor_mul(eg, e_pack, _bcast(G_bf, GQ))
                nc.vector.tensor_reduce(
                    wsum, eg, axis=mybir.AxisListType.XY, op=ALU.add
                )
                rsum = pack_pool.tile([128, 1], f32)
                nc.vector.reciprocal(rsum, esum)
                outcol = pack_pool.tile([128, 1], f32)
                nc.vector.tensor_mul(outcol, wsum, rsum)
                nc.sync.dma_start(
                    out=bass.AP(
                        tensor=out_t, offset=128 * pack, ap=[[1, 128], [1, 1]]
                    ),
                    in_=outcol,
                )

    nc.compile()
    return nc


_CACHED = {}


def _get_program(reps: int = 1):
    if reps not in _CACHED:
        _CACHED[reps] = build_program(reps)
    return _CACHED[reps]


def _host_consts(inputs):
    w1 = np.asarray(inputs["w1"], np.float32)
    b1 = np.asarray(inputs["b1"], np.float32)
    w2 = np.asarray(inputs["w2"], np.float32)
    b2 = np.asarray(inputs["b2"], np.float32)
    w3 = np.asarray(inputs["w3"], np.float32)

    W_A = w1[0:128] + w1[256:384]
    W_C = w1[128:256] - w1[256:384]
    W_P = w1[384:512]

    gram = (W_P.T @ W_P).astype(np.float64)
    M_u = (W_P @ np.linalg.solve(gram, W_A.T.astype(np.float64))).astype(
        np.float32
    )

    # sigmoid -> tanh folding: h1 = (1+t1)/2 with t1 = tanh(z1/2), so
    # z2 = w2.T h1 + b2 = (w2/2).T t1 + (b2 + w2.T 1 / 2); same for layer 3.
    w2half = 0.5 * w2
    b2eff = b2 + 0.5 * w2.sum(axis=0)
    w3half = 0.5 * w3[:, 0]

    cbf = np.zeros((128, C_END), np.float32)
    cbf[:, C_WP : C_WP + 80] = W_P
    cbf[:, C_WC : C_WC + 80] = W_C
    cbf[:, C_MU : C_MU + 128] = M_u.T
    cbf[0:80, C_W2 : C_W2 + 64] = np.concatenate(
        [w2half, np.zeros((80, 24), np.float32)], axis=1
    )
    cbf[0:40, C_W3 : C_W3 + 32] = w3half[:, None]
    cbf[64:104, C_W3B : C_W3B + 32] = w3half[:, None]
    # Msum[p, i] = 1 iff p % 32 == i  (strip-sum)
    p = np.arange(128)
    cbf[:, C_MS : C_MS + 32] = (p[:, None] % 32 == np.arange(32)[None, :])
    # per-quad penalty lhsT: rows j 0:4 -> 0 at in-strip row i == 4g+j of
    # strip s (group g = 4*quad + s), else -BIG; row 4 = +BIG (mask fold)
    for qd, col in ((0, C_P0), (1, C_P1)):
        p01 = np.zeros((128, 512), np.float32)
        for s in range(4):
            g = 4 * qd + s
            for j in range(GQ):
                i = 4 * g + j
                p01[32 * s + i, j * F : (j + 1) * F] = 1.0
        cbf[:, col : col + 512] = p01

    cf32 = np.zeros((128, 2), np.float32)
    cf32[0:80, 0] = 0.5 * b1
    cf32[0:40, 1] = 0.5 * b2eff
    cf32[64:104, 1] = 0.5 * b2eff

    return (
        np.ascontiguousarray(cbf.astype(ml_dtypes.bfloat16)),
        np.ascontiguousarray(cf32),
    )


def _make_in_maps(inputs):
    query = np.ascontiguousarray(np.asarray(inputs["query"], np.float32))
    facts = np.ascontiguousarray(np.asarray(inputs["facts"], np.float32))
    mask = np.ascontiguousarray(np.asarray(inputs["mask"], np.int32))
    cbf, cf32 = _host_consts(inputs)

    in_maps = []
    for c in range(N_CORES):
        sl = slice(c * BPC, (c + 1) * BPC)
        in_maps.append(
            {
                "query": np.ascontiguousarray(query[sl]),
                "facts": np.ascontiguousarray(facts[sl]),
                "mask": np.ascontiguousarray(mask[sl]),
                "cbf": cbf,
                "cf32": cf32,
            }
        )
    return in_maps


def run_traced(inputs, trace=False, reps=1):
    nc = _get_program(reps)
    res = run_bass_kernel_spmd(
        nc, _make_in_maps(inputs), core_ids=list(range(N_CORES)), trace=trace
    )
    out = np.concatenate(
        [res.results[c]["out"] for c in range(N_CORES)], axis=0
    )
    return out.astype(np.float32), res.exec_time_ns


def kernel(**inputs) -> np.ndarray:
    out, _ = run_traced(inputs, trace=False)
    return out
